# revision 1
# baseline (speedup 1.0000x reference)
"""Trainium2 Bass kernel for nn_MenuLoss_7713761264358.

Strategy (data parallel over 8 NeuronCores, 64 batch elements each):

The reference loss is dominated by soft-gaussian one-hot lookups
exp(-(x - i)^2 / 0.01) against the [223, 19] foods table.  Because every
query x is (to fp32 precision) an exact integer in [0, 222] -- pred ids
after round+mask, true ids by construction -- the gaussians are exact
one-hot selectors and every lookup collapses to a row gather data[x, :].
(Validated on host: full-decomposition rel err ~2.8e-7 vs the reference.)

Per-core pipeline:
  1. DVE computes pred indices: round-half-even via the 2^23 magic-number
     trick, then the >222.5 -> 0 mask, cast to int16.
  2. GPSIMD ap_gather does the table lookups.  Table columns sit on
     partitions (16 column slots per 16-partition GPSIMD core group, one
     independent 8-batch token stream per group), tokens on the free dim,
     so per-batch sums become free-dim strided reductions.  ap_gather
     costs ~27ns per index (measured), so the 14 binary table columns are
     packed in pairs (lo + 65536*hi; batch sums stay < 2^24 so fp32 sums
     are exact and unpack losslessly) -- one gather per id stream covers
     all 19 columns.
  3. PE broadcasts per-token amounts across partitions (rank-8 0/1 matmul)
     so DVE can form amount-weighted products, then DVE tensor_reduce
     produces per-batch / per-day / per-meal sums.
  4. ACT handles tanh/relu/exp/square/abs with fused accumulation (the
     tanh/relu penalty terms are linear in global sums, so they are
     computed directly on the gather-layout tiles).
  5. Final per-batch scalar math on [128, 8] tiles; per-class constant
     mask-weight vectors fold the batch means and class scales, and a
     ones-vector matmul contracts everything to one scalar per core.
Host work is layout-only: shard the batch across cores, de-interleave
id/amount, pre-permute ids into the gather's wrapped index layout, pack
the constant table, and sum the 8 per-core partial losses (all loss
terms end in batch means, so the cross-core reduction is a plain sum).
"""

import numpy as np

import concourse.bass as bass
import concourse.tile as tile
from concourse import bacc, mybir

AF = mybir.ActivationFunctionType
OP = mybir.AluOpType
AX = mybir.AxisListType
F32 = mybir.dt.float32
I16 = mybir.dt.int16

NCORES = 8
BG = 512            # global batch
BL = BG // NCORES   # 64 batches per core
S = 168             # slots per batch (7 days * 3 meals * 8 foods)
NG = 8              # token streams per core (one per 16-partition group)
NB = BL // NG       # 8 batches per stream
L = NB * S          # 1344 tokens per stream (gather num_idxs)
W = L // 16         # 84 idx columns in the wrapped idx layout
NH = L // 2         # half-stream split for gather pipelining

MAGIC = 8388608.0   # 2^23: (x + MAGIC) - MAGIC == round-half-even(x) for 0<=x<2^22
PKS = 65536.0       # packing scale for binary column pairs
ZCONST = 3000.0 * 504.0 / 8.0  # per-core constant part of the zeros penalty

# cstx column map (per-class mask weights, see make_const_inputs)
C_NUT, C_PREF, C_ALO, C_AHI, C_ILO, C_IHI, C_MEAL, C_VAR, C_ONE = range(9)
CSTW = 16           # cstx mask columns (padded), block matrix follows


def _build(tc, piw, tiw, pa, ta, tabs, cstx, out):
    import contextlib

    nc = tc.nc
    from concourse import library_config

    with contextlib.ExitStack() as ctx:
        sb = ctx.enter_context(tc.tile_pool(name="sb", bufs=1))
        ps = ctx.enter_context(tc.tile_pool(name="ps", bufs=1, space="PSUM"))

        # ---- constant tables ----
        tabs_s = sb.tile([128, 223], F32, tag="tabs_s")
        nc.sync.dma_start(out=tabs_s[:], in_=tabs)
        cstx_s = sb.tile([128, CSTW + 128], F32, tag="cstx_s")
        nc.sync.dma_start(out=cstx_s[:], in_=cstx)
        blk_s = cstx_s[0:8, CSTW:CSTW + 128]

        def cw(col):
            return cstx_s[:, col:col + 1]

        # ---- wrapped-layout ids (host pre-permuted) ----
        # Stream g covers local batches 8g..8g+7; stream token l = 168*b + s.
        # ap_gather unwraps indices as idx[l] = idxtile[16g + l%16, l//16].
        pidw = sb.tile([128, W], F32, tag="pidw")
        nc.sync.dma_start(out=pidw[:], in_=piw)
        tidw = sb.tile([128, W], F32, tag="tidw")
        nc.sync.dma_start(out=tidw[:], in_=tiw)

        # ---- pred index compute: round-half-even, mask >222.5 to 0 ----
        kpw = sb.tile([128, W], F32, tag="kpw")
        nc.vector.tensor_scalar(
            out=kpw[:], in0=pidw[:], scalar1=MAGIC, scalar2=MAGIC,
            op0=OP.add, op1=OP.subtract,
        )
        ipw = sb.tile([128, W], F32, tag="ipw")
        nc.vector.scalar_tensor_tensor(
            out=ipw[:], in0=kpw[:], scalar=222.5, in1=kpw[:],
            op0=OP.is_le, op1=OP.mult,
        )
        idxp = sb.tile([128, W], I16, tag="idxp")
        nc.vector.tensor_copy(out=idxp[:], in_=ipw[:])
        idxt = sb.tile([128, W], I16, tag="idxt")
        nc.vector.tensor_copy(out=idxt[:], in_=tidw[:])

        # The ap_gather ISA encoding carries at most ONE sync wait.  Pre-warm
        # the Pool engine's vector clock with dummy Pool-engine DMA reads of
        # every gather input so the gathers themselves need no waits.
        scr_a = sb.tile([1, 1], F32, tag="scr_a")
        nc.gpsimd.dma_start(out=scr_a[:], in_=tabs_s[0:1, 0:1])
        scr_i = sb.tile([1, 1], I16, tag="scr_i")
        nc.gpsimd.dma_start(out=scr_i[:], in_=idxt[0:1, 0:1])
        scr_p = sb.tile([1, 1], I16, tag="scr_p")
        nc.gpsimd.dma_start(out=scr_p[:], in_=idxp[0:1, 0:1])

        nc.gpsimd.load_library(library_config.ap_gather)

        # ---- gathers: out[p, l] = table[p, idx_g(l)], split in halves so
        # DVE consumers pipeline under the (dominant) gather time ----
        gap = sb.tile([128, L], F32, tag="gap")
        gat = sb.tile([128, L], F32, tag="gat")
        for h in range(2):
            for (g_t, idx) in ((gap, idxp), (gat, idxt)):
                nc.gpsimd.ap_gather(
                    out_ap=g_t[:, h * NH:(h + 1) * NH],
                    in_ap=tabs_s[:],
                    idxs_ap=idx[:, h * (W // 2):(h + 1) * (W // 2)],
                    channels=128, num_elems=223, d=1, num_idxs=NH,
                )

        # ---- amounts in stream layout + partition broadcast via matmul ----
        amp = sb.tile([8, L], F32, tag="amp")
        nc.sync.dma_start(out=amp[:], in_=pa)
        amt = sb.tile([8, L], F32, tag="amt")
        nc.sync.dma_start(out=amt[:], in_=ta)

        # PE also has a tight sync-wait budget: pre-warm its vector clock
        # with 1x1 dummy matmuls, one DMA dependency each.
        scr_m = ps.tile([1, 3], F32, tag="scr_m")
        for i, til in enumerate((cstx_s, amp, amt)):
            nc.tensor.matmul(
                scr_m[:, i:i + 1], til[0:1, 0:1], til[0:1, 0:1],
                start=True, stop=True,
            )

        ampp = ps.tile([128, L], F32, tag="ampp")
        amtp = ps.tile([128, L], F32, tag="amtp")
        for (src, dst) in ((amp, ampp), (amt, amtp)):
            for c0 in range(0, L, 512):
                c1 = min(c0 + 512, L)
                nc.tensor.matmul(
                    dst[:, c0:c1], blk_s, src[:, c0:c1],
                    start=True, stop=True,
                )

        # ---- products and per-batch reductions (split by gather half) ----
        prdp = sb.tile([128, L], F32, tag="prdp")
        prdt = sb.tile([128, L], F32, tag="prdt")

        def red(out_ap, in_ap, axis=AX.X):
            nc.vector.tensor_reduce(out=out_ap, in_=in_ap, axis=axis, op=OP.add)

        nutp = sb.tile([128, NB], F32, tag="nutp")
        nutt = sb.tile([128, NB], F32, tag="nutt")
        hap = sb.tile([128, NB], F32, tag="hap")
        hat = sb.tile([128, NB], F32, tag="hat")
        dayp = sb.tile([128, NB * 7], F32, tag="dayp")
        mealp = sb.tile([128, NB * 3], F32, tag="mealp")
        mealt = sb.tile([128, NB * 3], F32, tag="mealt")

        for h in range(2):
            hb = slice(h * (NB // 2), (h + 1) * (NB // 2))  # batches of half
            hc = slice(h * NH, (h + 1) * NH)                # token cols
            for (g_t, a_t, prd, nut, hsum, meal) in (
                (gap, ampp, prdp, nutp, hap, mealp),
                (gat, amtp, prdt, nutt, hat, mealt),
            ):
                nc.vector.tensor_tensor(
                    out=prd[:, hc], in0=g_t[:, hc], in1=a_t[:, hc], op=OP.mult
                )
                pb = prd[:, hc].rearrange("p (b s) -> p b s", s=S)
                gb = g_t[:, hc].rearrange("p (b s) -> p b s", s=S)
                red(nut[:, hb], pb)
                red(hsum[:, hb], gb)
                red(
                    meal[:, h * 12:(h + 1) * 12].rearrange(
                        "p (b m) -> p b m", m=3),
                    pb.rearrange("p b (d m f) -> p b m d f", d=7, m=3),
                    axis=AX.XY,
                )
            red(
                dayp[:, h * 28:(h + 1) * 28].rearrange("p (b d) -> p b d", d=7),
                prdp[:, hc].rearrange("p (b s) -> p b s", s=S
                                      ).rearrange("p b (d u) -> p b d u", d=7),
            )

        # ---- tanh / relu penalties (linear in global sums -> any layout) ----
        th1 = sb.tile([128, W], F32, tag="th1")
        st1 = sb.tile([128, 1], F32, tag="st1")
        nc.scalar.activation(
            out=th1[:], in_=pidw[:], func=AF.Tanh, scale=2.0, accum_out=st1[:]
        )
        th2 = sb.tile([8, L], F32, tag="th2")
        st2 = sb.tile([8, 1], F32, tag="st2")
        nc.scalar.activation(
            out=th2[:], in_=amp[:], func=AF.Tanh, scale=2.0, accum_out=st2[:]
        )
        rl1 = sb.tile([128, W], F32, tag="rl1")
        srel = sb.tile([128, 1], F32, tag="srel")
        cm222 = sb.tile([128, 1], F32, tag="cm222")
        nc.vector.memset(cm222[:], -222.0)
        nc.scalar.activation(
            out=rl1[:], in_=pidw[:], func=AF.Relu, bias=cm222[:], scale=1.0,
            accum_out=srel[:],
        )

        # ---- unpack the paired binary-column sums: S = lo + 65536*hi ----
        def unpack(tag, s_t):
            t1 = sb.tile([128, NB], F32, tag=tag + "_t1")
            nc.vector.tensor_scalar(
                out=t1[:], in0=s_t[:], scalar1=1.0 / PKS, scalar2=MAGIC,
                op0=OP.mult, op1=OP.add,
            )
            hi = sb.tile([128, NB], F32, tag=tag + "_hi")
            nc.vector.tensor_scalar(
                out=hi[:], in0=t1[:], scalar1=MAGIC, scalar2=None,
                op0=OP.subtract,
            )
            lo = sb.tile([128, NB], F32, tag=tag + "_lo")
            nc.vector.scalar_tensor_tensor(
                out=lo[:], in0=hi[:], scalar=-PKS, in1=s_t[:],
                op0=OP.mult, op1=OP.add,
            )
            return lo, hi

        lop, hip = unpack("up", hap)
        lot, hit = unpack("ut", hat)

        # ---- final per-batch math ----
        def sub(tag, a, b, shape):
            d = sb.tile(shape, F32, tag=tag)
            nc.vector.tensor_tensor(out=d[:], in0=a[:], in1=b[:], op=OP.subtract)
            return d

        def huber(tag, d, scale, shape):
            # huber(scale*d) = m*(A - 0.5m), A = |scale*d|, m = min(A, 1)
            a_t = sb.tile(shape, F32, tag=tag + "_a")
            nc.scalar.activation(out=a_t[:], in_=d[:], func=AF.Abs, scale=scale)
            m_t = sb.tile(shape, F32, tag=tag + "_m")
            nc.vector.tensor_scalar(
                out=m_t[:], in0=a_t[:], scalar1=1.0, scalar2=None, op0=OP.min
            )
            t_t = sb.tile(shape, F32, tag=tag + "_t")
            nc.vector.scalar_tensor_tensor(
                out=t_t[:], in0=m_t[:], scalar=-0.5, in1=a_t[:],
                op0=OP.mult, op1=OP.add,
            )
            h_t = sb.tile(shape, F32, tag=tag + "_h")
            nc.vector.tensor_tensor(out=h_t[:], in0=m_t[:], in1=t_t[:], op=OP.mult)
            return h_t

        hn = huber("hn", sub("dn", nutp, nutt, [128, NB]), 1.0 / 700.0, [128, NB])
        hilo = huber(
            "hl", sub("dl", lop, lot, [128, NB]), 1.0, [128, NB])
        hihi = huber(
            "hh", sub("dh", hip, hit, [128, NB]), 1.0, [128, NB])
        hm = huber(
            "hm", sub("dm", mealp, mealt, [128, NB * 3]), 1.0 / 700.0,
            [128, NB * 3])

        # prefs: exp(10*G - 1680) * (168-P)^2 ; allergens: exp(-10*G) * P^2
        cm1680 = sb.tile([128, 1], F32, tag="cm1680")
        nc.vector.memset(cm1680[:], -1680.0)

        def prefall(tag, p_t, g_t):
            gc = sb.tile([128, NB], F32, tag=tag + "_gc")
            nc.vector.tensor_scalar(
                out=gc[:], in0=g_t[:], scalar1=168.0, scalar2=None, op0=OP.min
            )
            e1 = sb.tile([128, NB], F32, tag=tag + "_e1")
            nc.scalar.activation(
                out=e1[:], in_=gc[:], func=AF.Exp, scale=10.0, bias=cm1680[:]
            )
            p1 = sb.tile([128, NB], F32, tag=tag + "_p1")
            nc.vector.tensor_scalar(
                out=p1[:], in0=p_t[:], scalar1=-1.0, scalar2=168.0,
                op0=OP.mult, op1=OP.add,
            )
            q1 = sb.tile([128, NB], F32, tag=tag + "_q1")
            nc.scalar.activation(out=q1[:], in_=p1[:], func=AF.Square)
            v1 = sb.tile([128, NB], F32, tag=tag + "_v1")
            nc.vector.tensor_tensor(out=v1[:], in0=e1[:], in1=q1[:], op=OP.mult)
            # clamp below: junk lanes can unpack negative -> exp(+inf) -> NaN
            gp_t = sb.tile([128, NB], F32, tag=tag + "_gp")
            nc.vector.tensor_scalar(
                out=gp_t[:], in0=g_t[:], scalar1=0.0, scalar2=None, op0=OP.max
            )
            e2 = sb.tile([128, NB], F32, tag=tag + "_e2")
            nc.scalar.activation(out=e2[:], in_=gp_t[:], func=AF.Exp, scale=-10.0)
            q2 = sb.tile([128, NB], F32, tag=tag + "_q2")
            nc.scalar.activation(out=q2[:], in_=p_t[:], func=AF.Square)
            v2 = sb.tile([128, NB], F32, tag=tag + "_v2")
            nc.vector.tensor_tensor(out=v2[:], in0=e2[:], in1=q2[:], op=OP.mult)
            return v1, v2

        v1lo, v2lo = prefall("plo", lop, lot)
        v1hi, v2hi = prefall("phi", hip, hit)

        # day-level variance: var = S2/7 - (S1/700)^2 with cal = day/100
        s1 = sb.tile([128, NB], F32, tag="s1")
        red(s1[:], dayp[:].rearrange("p (b d) -> p b d", d=7))
        sq = sb.tile([128, NB * 7], F32, tag="sq")
        nc.scalar.activation(out=sq[:], in_=dayp[:], func=AF.Square, scale=0.01)
        s2 = sb.tile([128, NB], F32, tag="s2")
        red(s2[:], sq[:].rearrange("p (b d) -> p b d", d=7))
        mu2 = sb.tile([128, NB], F32, tag="mu2")
        nc.vector.scalar_tensor_tensor(
            out=mu2[:], in0=s1[:], scalar=1.0 / 490000.0, in1=s1[:],
            op0=OP.mult, op1=OP.mult,
        )
        varb = sb.tile([128, NB], F32, tag="varb")
        nc.vector.scalar_tensor_tensor(
            out=varb[:], in0=s2[:], scalar=1.0 / 7.0, in1=mu2[:],
            op0=OP.mult, op1=OP.subtract,
        )

        # ---- mask-weight accumulate + final contraction ----
        acc = sb.tile([128, NB + 2], F32, tag="acc")
        nc.vector.tensor_scalar_mul(out=acc[:, 0:NB], in0=hn[:], scalar1=cw(C_NUT))
        for (val, col) in (
            (v1lo, C_PREF), (v1hi, C_PREF), (v2lo, C_ALO), (v2hi, C_AHI),
            (hilo, C_ILO), (hihi, C_IHI), (varb, C_VAR),
        ):
            nc.vector.scalar_tensor_tensor(
                out=acc[:, 0:NB], in0=val[:], scalar=cw(col),
                in1=acc[:, 0:NB], op0=OP.mult, op1=OP.add,
            )
        nc.vector.tensor_scalar_mul(
            out=acc[:, NB:NB + 1], in0=st1[:], scalar1=-2.0 * 3000.0 / 512.0
        )
        nc.vector.tensor_scalar_mul(
            out=acc[:, NB + 1:NB + 2], in0=srel[:], scalar1=1.0 / 512.0
        )
        accm = sb.tile([128, NB * 3], F32, tag="accm")
        nc.vector.tensor_scalar_mul(out=accm[:], in0=hm[:], scalar1=cw(C_MEAL))
        st2w = sb.tile([8, 1], F32, tag="st2w")
        nc.vector.tensor_scalar_mul(
            out=st2w[:], in0=st2[:], scalar1=-3000.0 / 512.0
        )

        nf = (NB + 2) + NB * 3 + 1  # 35
        fps = ps.tile([1, nf], F32, tag="fps")
        nc.tensor.matmul(
            fps[:, 0:NB + 2], cw(C_ONE), acc[:], start=True, stop=True
        )
        nc.tensor.matmul(
            fps[:, NB + 2:NB + 2 + NB * 3], cw(C_ONE), accm[:],
            start=True, stop=True,
        )
        nc.tensor.matmul(
            fps[:, nf - 1:nf], cstx_s[0:8, C_ONE:C_ONE + 1], st2w[:],
            start=True, stop=True,
        )
        loss_t = sb.tile([1, 1], F32, tag="loss_t")
        nc.vector.tensor_reduce(out=loss_t[:], in_=fps[:], axis=AX.X, op=OP.add)
        lossf = sb.tile([1, 1], F32, tag="lossf")
        nc.vector.tensor_scalar_add(out=lossf[:], in0=loss_t[:], scalar1=ZCONST)
        # With few input DMAs on SP, the out DMA lands on a fresh HW queue:
        # no queue-order wait, only the DVE data wait (1-wait DMA budget).
        nc.sync.dma_start(out=out, in_=lossf[:])


def build_program():
    nc = bacc.Bacc("TRN2", target_bir_lowering=False, num_devices=NCORES)
    piw = nc.dram_tensor("piw", [128, W], F32, kind="ExternalInput")
    tiw = nc.dram_tensor("tiw", [128, W], F32, kind="ExternalInput")
    pa = nc.dram_tensor("pa", [8, L], F32, kind="ExternalInput")
    ta = nc.dram_tensor("ta", [8, L], F32, kind="ExternalInput")
    tabs = nc.dram_tensor("tabs", [128, 223], F32, kind="ExternalInput")
    cstx = nc.dram_tensor("cstx", [128, CSTW + 128], F32, kind="ExternalInput")
    out = nc.dram_tensor("o", [1, 1], F32, kind="ExternalOutput")
    with tile.TileContext(nc) as tc:
        _build(
            tc, piw.ap(), tiw.ap(), pa.ap(), ta.ap(),
            tabs.ap(), cstx.ap(), out.ap(),
        )
    nc.compile()
    return nc


def wrap_ids(ids_flat):
    """[64*168] flat ids -> [128, 84] wrapped gather-idx layout."""
    arr = np.ascontiguousarray(ids_flat, dtype=np.float32).reshape(NG, W, 16)
    # token l of stream g sits at [16g + l%16, l//16]
    return arr.transpose(0, 2, 1).reshape(128, W).copy()


def make_const_inputs(data):
    """Host-side constant tables shared by all cores."""
    data = np.asarray(data, dtype=np.float32)
    # packed column table: 16 slots per group
    pk = np.zeros((223, 16), np.float32)
    pk[:, 0:5] = data[:, 0:5]
    pairs = [(5, 6), (7, 8), (9, 10), (11, 12), (13, None),
             (14, 15), (16, 17), (18, None)]
    for j, (a, b) in enumerate(pairs):
        col = data[:, a].astype(np.float64)
        if b is not None:
            col = col + PKS * data[:, b].astype(np.float64)
        pk[:, 5 + j] = col.astype(np.float32)
    tabs = np.zeros((128, 223), np.float32)
    for g in range(NG):
        tabs[16 * g:16 * g + 16] = pk.T

    blk = np.zeros((8, 128), np.float32)
    for g in range(8):
        blk[g, 16 * g:16 * g + 16] = 1.0
    c = np.arange(128) % 16
    cstx = np.zeros((128, CSTW + 128), np.float32)
    w_hub = 1.0 / (100.0 * 512.0)
    w_pa = 100.0 / 512.0
    cstx[:, C_NUT] = (c < 5) * w_hub
    cstx[:, C_PREF] = (c == 5) * w_pa
    cstx[:, C_ALO] = ((c >= 6) & (c <= 9)) * w_pa
    cstx[:, C_AHI] = ((c >= 6) & (c <= 8)) * w_pa
    cstx[:, C_ILO] = ((c >= 10) & (c <= 12)) * w_hub
    cstx[:, C_IHI] = ((c >= 10) & (c <= 11)) * w_hub
    cstx[:, C_MEAL] = (c == 0) * w_hub
    cstx[:, C_VAR] = (c == 0) / 512.0
    cstx[:, C_ONE] = 1.0
    cstx[0:8, CSTW:CSTW + 128] = blk
    return tabs, cstx


def make_in_maps(y_pred, y, data):
    y_pred = np.asarray(y_pred, dtype=np.float32)
    y = np.asarray(y, dtype=np.float32)
    tabs, cstx = make_const_inputs(data)
    in_maps = []
    for core in range(NCORES):
        sl = slice(core * BL, (core + 1) * BL)

        def flat(arr, comp):
            return np.ascontiguousarray(arr[sl, ..., comp], dtype=np.float32
                                        ).reshape(-1)

        in_maps.append({
            "piw": wrap_ids(flat(y_pred, 0)),
            "pa": flat(y_pred, 1).reshape(NG, L),
            "tiw": wrap_ids(flat(y, 0)),
            "ta": flat(y, 1).reshape(NG, L),
            "tabs": tabs, "cstx": cstx,
        })
    return in_maps


_NC_CACHE = None


def _get_nc():
    global _NC_CACHE
    if _NC_CACHE is None:
        _NC_CACHE = build_program()
    return _NC_CACHE


def run_on_hw(y_pred, y, data, **kwargs):
    from concourse.bass_utils import run_bass_kernel_spmd

    nc = _get_nc()
    in_maps = make_in_maps(y_pred, y, data)
    res = run_bass_kernel_spmd(
        nc, in_maps, core_ids=list(range(NCORES)), **kwargs
    )
    parts = [r["o"][0, 0] for r in res.results]
    return np.float32(np.sum(np.asarray(parts, dtype=np.float32))), res


def kernel(y_pred, y, data):
    return run_on_hw(y_pred, y, data)[0]



# revision 9
# speedup vs baseline: 2.1457x; 2.1457x over previous
"""Trainium2 Bass kernel for nn_MenuLoss_7713761264358.

Strategy (data parallel over 8 NeuronCores, 64 batch elements each):

Every id lookup in the reference collapses to a row gather data[x, :]
(ids are exact integers after round+mask).  Instead of GPSIMD ap_gather
(~27ns/idx, ~75us), the lookup runs as a bf16 one-hot matmul on the PE:

  1. Ids arrive 16x-replicated per 16-partition group (partition (g, j)
     holds stream g's ids).  DVE rounds+masks pred ids, then builds 14
     "id planes" oh_k[(g,j), s] = [x_{g,s} == 16k + j] via tensor_scalar
     is_equal against a per-partition iota column (bf16, 4x perf mode).
  2. PE accumulates 14 block-diagonal matmuls (lhsT_k[(g,j),(g,c)] =
     packed_table[16k+j, c] per group) into vals[(g,c), s] PSUM —
     exactly the table lookup for all 16 packed columns at once.
     Binary column pairs pack as lo + 248*hi ({0,1,248,249} all
     bf16-exact; per-batch lo-sums <= 168 < 248 so sums unpack exactly).
  3. ACT copies vals to SBUF bf16; DVE multiplies by an amounts-or-ones
     tile (amounts for the 5 continuous slots, 1.0 for binary slots) so
     one product + one food-axis reduce yields both the nutrition sums
     and the binary count sums; small strided reduces produce per-batch
     / per-day / per-meal sums.
  4. ACT handles tanh/relu/exp/square/abs (penalties, huber, prefs).
  5. All per-batch terms are written into one valcat tile, multiplied by
     a host-built per-(partition, column) mask-weight tile, and
     contracted to a scalar with a single ones-column matmul.
Host work is layout-only: de-interleave ids/amounts, replicate across
partition groups, pack the constant tables, sum 8 per-core partials.
"""

import numpy as np
import ml_dtypes

import concourse.bass as bass
import concourse.tile as tile
from concourse import bacc, mybir

AF = mybir.ActivationFunctionType
OP = mybir.AluOpType
AX = mybir.AxisListType
F32 = mybir.dt.float32
BF16 = mybir.dt.bfloat16
BFNP = ml_dtypes.bfloat16

NCORES = 8
BG = 512            # global batch
BL = BG // NCORES   # 64 batches per core
S = 168             # slots per batch (7 days * 3 meals * 8 foods)
NG = 8              # streams (one per 16-partition group)
NB = BL // NG       # 8 batches per stream
L = NB * S          # 1344 tokens per stream per id-type
L2 = 2 * L          # true tokens ‖ pred tokens
NK = 14             # id planes: 14*16 = 224 >= 223
SIG = 248.0         # binary pair packing scale: lo + 248*hi
MAGIC = 8388608.0   # 2^23 round-half-even trick
ZCONST = 3000.0 * 504.0 / 8.0   # per-core constant part of zeros penalty
CHUNK = 448         # PE moving-operand chunk (1344 = 3*448)

W_HUB = 1.0 / (100.0 * 512.0)
W_PA = 100.0 / 512.0
NV = 92             # valcat columns (91 used + pad to even)


def _build(tc, xp, xt, amtb, am8, wts, iotab, mcat, out):
    import contextlib

    nc = tc.nc

    with contextlib.ExitStack() as ctx:
        sb = ctx.enter_context(tc.tile_pool(name="sb", bufs=1))
        ps = ctx.enter_context(tc.tile_pool(name="ps", bufs=1, space="PSUM"))

        # ---- input DMAs, spread across engine queues ----
        # sync: true ids first (unblocks DVE), then raw pred ids
        xcat = sb.tile([128, L2], BF16, tag="xcat")
        nc.sync.dma_start(out=xcat[:, 0:L], in_=xt)
        xp_s = sb.tile([128, L], F32, tag="xp_s")
        nc.sync.dma_start(out=xp_s[:], in_=xp)
        # scalar queue: amounts-or-ones tile
        amtb_s = sb.tile([128, L2], BF16, tag="amtb_s")
        nc.scalar.dma_start(out=amtb_s[:], in_=amtb)
        # gpsimd queue: iota first (unblocks DVE), then lookup weights
        iotab_s = sb.tile([128, NK], F32, tag="iotab_s")
        nc.gpsimd.dma_start(out=iotab_s[:], in_=iotab)
        wts_s = sb.tile([128, NK * 128 + 1], BF16, tag="wts_s")
        nc.gpsimd.dma_start(out=wts_s[:], in_=wts)
        am8_s = sb.tile([8, L], F32, tag="am8_s")
        nc.gpsimd.dma_start(out=am8_s[:], in_=am8)
        mcat_s = sb.tile([128, NV], F32, tag="mcat_s")
        nc.gpsimd.dma_start(out=mcat_s[:], in_=mcat)
        valcat = sb.tile([128, NV], F32, tag="valcat")
        nc.gpsimd.memset(valcat[:], 0.0)
        cm222 = sb.tile([128, 1], F32, tag="cm222")
        nc.gpsimd.memset(cm222[:], -222.0)
        cm1680 = sb.tile([128, 1], F32, tag="cm1680")
        nc.gpsimd.memset(cm1680[:], -1680.0)
        ones_t = sb.tile([128, 1], F32, tag="ones_t")
        nc.gpsimd.memset(ones_t[:], 1.0)

        # ---- id planes: oh_k[(g,j), s] = [x == 16k + j]  (bf16 4x) ----
        oh = [
            sb.tile([128, L2], BF16, name=f"oh{k}", tag=f"oh{k}")
            for k in range(NK)
        ]
        for k in range(NK):   # true half first: no preprocessing needed
            nc.vector.tensor_scalar(
                out=oh[k][:, 0:L], in0=xcat[:, 0:L],
                scalar1=iotab_s[:, k:k + 1], scalar2=None, op0=OP.is_equal,
            )
        # pred ids: round-half-even then mask >222.5 -> 0, write bf16
        kp = sb.tile([128, L], F32, tag="kp")
        nc.vector.tensor_scalar(
            out=kp[:], in0=xp_s[:], scalar1=MAGIC, scalar2=MAGIC,
            op0=OP.add, op1=OP.subtract,
        )
        nc.vector.scalar_tensor_tensor(
            out=xcat[:, L:L2], in0=kp[:], scalar=222.5, in1=kp[:],
            op0=OP.is_le, op1=OP.mult,
        )
        for k in range(NK):
            nc.vector.tensor_scalar(
                out=oh[k][:, L:L2], in0=xcat[:, L:L2],
                scalar1=iotab_s[:, k:k + 1], scalar2=None, op0=OP.is_equal,
            )

        # ---- PE: accumulate 14 block-diag lookup matmuls per chunk ----
        # PSUM bank = 512 fp32: halves at 1536-col offsets, chunks 512/512/320
        vals = ps.tile([128, 3072], F32, tag="vals")
        for h in range(2):          # 0 = true cols, 1 = pred cols
            for c0 in (0, 512, 1024):
                w = min(512, L - c0)
                for k in range(NK):
                    nc.tensor.matmul(
                        vals[:, h * 1536 + c0:h * 1536 + c0 + w],
                        wts_s[:, 128 * k:128 * (k + 1)],
                        oh[k][:, h * L + c0:h * L + c0 + w],
                        start=(k == 0), stop=(k == NK - 1),
                    )

        # ---- ACT penalties from raw ids/amounts (independent path) ----
        th1 = sb.tile([128, L], F32, tag="th1")
        nc.scalar.activation(
            out=th1[:], in_=xp_s[:], func=AF.Tanh, scale=2.0,
            accum_out=valcat[:, 88:89],
        )
        rl1 = sb.tile([128, L], F32, tag="rl1")
        nc.scalar.activation(
            out=rl1[:], in_=xp_s[:], func=AF.Relu, bias=cm222[:], scale=1.0,
            accum_out=valcat[:, 89:90],
        )
        th2 = sb.tile([8, L], F32, tag="th2")
        nc.scalar.activation(
            out=th2[:], in_=am8_s[:], func=AF.Tanh, scale=2.0,
            accum_out=valcat[0:8, 90:91],
        )

        # ---- per-half: ACT copy PSUM->SBUF bf16, products, food-reduce ----
        vals_sb = sb.tile([128, L2], BF16, tag="vals_sb")
        prdv = sb.tile([128, L2], BF16, tag="prdv")
        prd8 = sb.tile([128, 336], F32, tag="prd8")  # (h, b, d, m)
        for h in range(2):
            cs = slice(h * L, (h + 1) * L)
            nc.scalar.activation(
                out=vals_sb[:, cs], in_=vals[:, h * 1536:h * 1536 + L],
                func=AF.Copy, scale=1.0,
            )
            nc.vector.tensor_tensor(
                out=prdv[:, cs], in0=vals_sb[:, cs], in1=amtb_s[:, cs],
                op=OP.mult,
            )
            nc.vector.tensor_reduce(
                out=prd8[:, h * 168:(h + 1) * 168],
                in_=prdv[:, cs].rearrange("p (u f) -> p u f", f=8),
                axis=AX.X, op=OP.add,
            )

        # ---- second-stage reduces ----
        psums = sb.tile([128, 16], F32, tag="psums")  # (h, b)
        nc.vector.tensor_reduce(
            out=psums[:], in_=prd8[:].rearrange("p (hb u) -> p hb u", u=21),
            axis=AX.X, op=OP.add,
        )
        meal = sb.tile([128, 48], F32, tag="meal")    # (h, b, m)
        nc.vector.tensor_reduce(
            out=meal[:].rearrange("p (hb m) -> p hb m", m=3),
            in_=prd8[:].rearrange("p (hb d m) -> p hb m d", d=7, m=3),
            axis=AX.X, op=OP.add,
        )
        day = sb.tile([128, 56], F32, tag="day")      # (b, d) pred half
        nc.vector.tensor_reduce(
            out=day[:].rearrange("p (b d) -> p b d", d=7),
            in_=prd8[:, 168:336].rearrange("p (b d m) -> p b d m", d=7, m=3),
            axis=AX.X, op=OP.add,
        )

        # ---- day-level variance: var = s2/7 - (s1/700)^2, cal = day/100 ----
        sq = sb.tile([128, 56], F32, tag="sq")
        nc.scalar.activation(out=sq[:], in_=day[:], func=AF.Square, scale=0.01)
        s2 = sb.tile([128, 8], F32, tag="s2")
        nc.vector.tensor_reduce(
            out=s2[:], in_=sq[:].rearrange("p (b d) -> p b d", d=7),
            axis=AX.X, op=OP.add,
        )
        mu2 = sb.tile([128, 8], F32, tag="mu2")
        nc.vector.scalar_tensor_tensor(
            out=mu2[:], in0=psums[:, 8:16], scalar=1.0 / 490000.0,
            in1=psums[:, 8:16], op0=OP.mult, op1=OP.mult,
        )
        nc.vector.scalar_tensor_tensor(
            out=valcat[:, 80:88], in0=s2[:], scalar=1.0 / 7.0, in1=mu2[:],
            op0=OP.mult, op1=OP.subtract,
        )

        # ---- unpack binary sums: S = lo + 248*hi -> PG[(G lo|hi, P lo|hi)] ----
        # PG cols: 0:8 lot, 8:16 hit, 16:24 lop, 24:32 hip
        pg = sb.tile([128, 32], F32, tag="pg")
        t1 = sb.tile([128, 16], F32, tag="t1")
        nc.vector.tensor_scalar(
            out=t1[:], in0=psums[:], scalar1=1.0 / SIG, scalar2=MAGIC - 0.33871,
            op0=OP.mult, op1=OP.add,
        )
        hi_v = pg[:].rearrange("p (v q b) -> p v q b", v=2, q=2)[:, :, 1:2, :]
        lo_v = pg[:].rearrange("p (v q b) -> p v q b", v=2, q=2)[:, :, 0:1, :]
        nc.vector.tensor_scalar(
            out=hi_v, in0=t1[:], scalar1=MAGIC, scalar2=None, op0=OP.subtract,
        )
        nc.vector.scalar_tensor_tensor(
            out=lo_v, in0=hi_v, scalar=-SIG, in1=psums[:],
            op0=OP.mult, op1=OP.add,
        )
        g2 = pg[:, 0:16]   # gold (true):  lot ‖ hit
        p2 = pg[:, 16:32]  # pred:         lop ‖ hip

        # ---- huber terms -> valcat[0:48] ----
        # d1: nutrition diffs (8) ‖ meal diffs (24), scale 1/700
        d1 = sb.tile([128, 32], F32, tag="d1")
        nc.vector.tensor_tensor(
            out=d1[:, 0:8], in0=psums[:, 8:16], in1=psums[:, 0:8],
            op=OP.subtract,
        )
        nc.vector.tensor_tensor(
            out=d1[:, 8:32], in0=meal[:, 24:48], in1=meal[:, 0:24],
            op=OP.subtract,
        )
        d2 = sb.tile([128, 16], F32, tag="d2")
        nc.vector.tensor_tensor(
            out=d2[:], in0=p2, in1=g2, op=OP.subtract,
        )

        def huber(dst, d_ap, scale, w, tag):
            a_t = sb.tile([128, w], F32, tag=tag + "_a")
            nc.scalar.activation(out=a_t[:], in_=d_ap, func=AF.Abs, scale=scale)
            m_t = sb.tile([128, w], F32, tag=tag + "_m")
            nc.vector.tensor_scalar(
                out=m_t[:], in0=a_t[:], scalar1=1.0, scalar2=None, op0=OP.min
            )
            t_t = sb.tile([128, w], F32, tag=tag + "_t")
            nc.vector.scalar_tensor_tensor(
                out=t_t[:], in0=m_t[:], scalar=-0.5, in1=a_t[:],
                op0=OP.mult, op1=OP.add,
            )
            nc.vector.tensor_tensor(out=dst, in0=m_t[:], in1=t_t[:], op=OP.mult)

        huber(valcat[:, 0:32], d1[:], 1.0 / 700.0, 32, "h1")
        huber(valcat[:, 32:48], d2[:], 1.0, 16, "h2")

        # ---- pref/allergen terms -> valcat[48:80] ----
        gc = sb.tile([128, 16], F32, tag="gc")
        nc.vector.tensor_scalar(
            out=gc[:], in0=g2, scalar1=168.0, scalar2=None, op0=OP.min
        )
        e1 = sb.tile([128, 16], F32, tag="e1")
        nc.scalar.activation(
            out=e1[:], in_=gc[:], func=AF.Exp, scale=10.0, bias=cm1680[:]
        )
        p1 = sb.tile([128, 16], F32, tag="p1")
        nc.vector.tensor_scalar(
            out=p1[:], in0=p2, scalar1=-1.0, scalar2=168.0,
            op0=OP.mult, op1=OP.add,
        )
        q1 = sb.tile([128, 16], F32, tag="q1")
        nc.scalar.activation(out=q1[:], in_=p1[:], func=AF.Square)
        nc.vector.tensor_tensor(
            out=valcat[:, 48:64], in0=e1[:], in1=q1[:], op=OP.mult
        )
        gp = sb.tile([128, 16], F32, tag="gp")
        nc.vector.tensor_scalar(
            out=gp[:], in0=g2, scalar1=0.0, scalar2=None, op0=OP.max
        )
        e2 = sb.tile([128, 16], F32, tag="e2")
        nc.scalar.activation(out=e2[:], in_=gp[:], func=AF.Exp, scale=-10.0)
        q2 = sb.tile([128, 16], F32, tag="q2")
        nc.scalar.activation(out=q2[:], in_=p2, func=AF.Square)
        nc.vector.tensor_tensor(
            out=valcat[:, 64:80], in0=e2[:], in1=q2[:], op=OP.mult
        )

        # ---- weighted contraction: one mult + one ones-column matmul ----
        wv = sb.tile([128, NV], F32, tag="wv")
        nc.vector.tensor_tensor(
            out=wv[:], in0=valcat[:], in1=mcat_s[:], op=OP.mult
        )
        fps = ps.tile([1, NV], F32, tag="fps")
        nc.tensor.matmul(
            fps[:], ones_t[:], wv[:], start=True, stop=True,
        )
        loss_t = sb.tile([1, 1], F32, tag="loss_t")
        nc.vector.tensor_reduce(out=loss_t[:], in_=fps[:], axis=AX.X, op=OP.add)
        lossf = sb.tile([1, 1], F32, tag="lossf")
        nc.vector.tensor_scalar_add(out=lossf[:], in0=loss_t[:], scalar1=ZCONST)
        nc.sync.dma_start(out=out, in_=lossf[:])


def build_program():
    nc = bacc.Bacc("TRN2", target_bir_lowering=False, num_devices=NCORES)
    xp = nc.dram_tensor("xp", [128, L], F32, kind="ExternalInput")
    xt = nc.dram_tensor("xt", [128, L], BF16, kind="ExternalInput")
    amtb = nc.dram_tensor("amtb", [128, L2], BF16, kind="ExternalInput")
    am8 = nc.dram_tensor("am8", [8, L], F32, kind="ExternalInput")
    wts = nc.dram_tensor("wts", [128, NK * 128 + 1], BF16, kind="ExternalInput")
    iotab = nc.dram_tensor("iotab", [128, NK], F32, kind="ExternalInput")
    mcat = nc.dram_tensor("mcat", [128, NV], F32, kind="ExternalInput")
    out = nc.dram_tensor("o", [1, 1], F32, kind="ExternalOutput")
    with tile.TileContext(nc) as tc:
        _build(
            tc, xp.ap(), xt.ap(), amtb.ap(), am8.ap(), wts.ap(),
            iotab.ap(), mcat.ap(), out.ap(),
        )
    nc.compile()
    return nc


def make_const_inputs(data):
    """Host-side constant tables shared by all cores (layout only)."""
    data = np.asarray(data, dtype=np.float32)
    # packed table [224, 16]: 5 cont cols, 7 sigma-packed binary pairs
    pk = np.zeros((224, 16), np.float32)
    pk[:223, 0:5] = data[:, 0:5]
    for u in range(7):
        pk[:223, 5 + u] = data[:, 5 + 2 * u] + SIG * data[:, 6 + 2 * u]
    # 14 block-diagonal lhsT planes + trailing ones column
    wts = np.zeros((128, NK * 128 + 1), BFNP)
    for k in range(NK):
        blk = np.zeros((128, 128), np.float32)
        e_k = pk[16 * k:16 * k + 16, :]
        for g in range(NG):
            blk[16 * g:16 * g + 16, 16 * g:16 * g + 16] = e_k
        wts[:, 128 * k:128 * (k + 1)] = blk.astype(BFNP)
    wts[:, NK * 128] = np.float32(1.0)
    # iota planes: value 16k + (p % 16)
    iotab = (
        16.0 * np.arange(NK)[None, :] + (np.arange(128) % 16)[:, None]
    ).astype(np.float32)
    # mask-weight tile [128, NV]
    c = np.arange(128) % 16
    m = np.zeros((128, NV), np.float32)
    m[:, 0:8] = ((c < 5) * W_HUB)[:, None]            # nutrition huber
    m[:, 8:32] = ((c == 0) * W_HUB)[:, None]          # meal huber
    m[:, 32:40] = (((c == 10) | (c == 11)) * W_HUB)[:, None]      # ingr lo
    m[:, 40:48] = (((c >= 9) & (c <= 11)) * W_HUB)[:, None]       # ingr hi
    m[:, 48:64] = ((c == 5) * W_PA)[:, None]          # prefs lo+hi
    m[:, 64:72] = (((c >= 6) & (c <= 9)) * W_PA)[:, None]         # alrg lo
    m[:, 72:80] = (((c >= 6) & (c <= 8)) * W_PA)[:, None]         # alrg hi
    m[:, 80:88] = ((c == 0) / 512.0)[:, None]         # day variance
    m[:, 88] = -2.0 * 3000.0 / (512.0 * 16.0)         # tanh(2*pid), 16x repl
    m[:, 89] = 1.0 / (512.0 * 16.0)                   # relu(pid-222), 16x
    m[:, 90] = 0.0
    m[0:8, 90] = -3000.0 / 512.0                      # tanh(2*pamt), compact
    return wts, iotab, m


def make_in_maps(y_pred, y, data):
    y_pred = np.asarray(y_pred, dtype=np.float32)
    y = np.asarray(y, dtype=np.float32)
    wts, iotab, mcat = make_const_inputs(data)
    in_maps = []
    for core in range(NCORES):
        sl = slice(core * BL, (core + 1) * BL)

        def streams(arr, comp):
            # [64, 7, 3, 8] -> [8 streams, 1344] (batch-major within stream)
            return np.ascontiguousarray(
                arr[sl, ..., comp], dtype=np.float32).reshape(NG, L)

        pid = streams(y_pred, 0)
        pam = streams(y_pred, 1)
        tid = streams(y, 0)
        tam = streams(y, 1)
        # replicate ids across the 16 partitions of each group
        xp = np.repeat(pid, 16, axis=0)                      # [128, L] f32
        xt = np.repeat(tid.astype(BFNP), 16, axis=0)         # [128, L] bf16
        # amounts-or-ones: slots c<5 amounts, else 1.0 ; true ‖ pred
        c16 = (np.arange(128) % 16)[:, None]
        amtb = np.concatenate(
            [np.repeat(tam, 16, axis=0), np.repeat(pam, 16, axis=0)], axis=1
        )
        amtb = np.where(c16 < 5, amtb, 1.0).astype(BFNP)     # [128, 2L]
        in_maps.append({
            "xp": xp, "xt": xt, "amtb": amtb,
            "am8": pam, "wts": wts, "iotab": iotab, "mcat": mcat,
        })
    return in_maps


_NC_CACHE = None


def _get_nc():
    global _NC_CACHE
    if _NC_CACHE is None:
        _NC_CACHE = build_program()
    return _NC_CACHE


def run_on_hw(y_pred, y, data, **kwargs):
    from concourse.bass_utils import run_bass_kernel_spmd

    nc = _get_nc()
    in_maps = make_in_maps(y_pred, y, data)
    res = run_bass_kernel_spmd(
        nc, in_maps, core_ids=list(range(NCORES)), **kwargs
    )
    parts = [r["o"][0, 0] for r in res.results]
    return np.float32(np.sum(np.asarray(parts, dtype=np.float32))), res


def kernel(y_pred, y, data):
    return run_on_hw(y_pred, y, data)[0]


# revision 12
# speedup vs baseline: 2.4025x; 1.1197x over previous
"""Trainium2 Bass kernel for nn_MenuLoss_7713761264358.

Strategy (data parallel over 8 NeuronCores, 64 batch elements each):

Every id lookup in the reference collapses to a row gather data[x, :]
(ids are exact integers after round+mask).  Instead of GPSIMD ap_gather
(~27ns/idx, ~75us), the lookup runs as a bf16 one-hot matmul on the PE:

  1. Ids arrive 16x-replicated per 16-partition group (partition (g, j)
     holds stream g's ids).  DVE rounds+masks pred ids, then builds 14
     "id planes" oh_k[(g,j), s] = [x_{g,s} == 16k + j] via tensor_scalar
     is_equal against a per-partition iota column (bf16, 4x perf mode).
  2. PE accumulates 14 block-diagonal matmuls (lhsT_k[(g,j),(g,c)] =
     packed_table[16k+j, c] per group) into vals[(g,c), s] PSUM —
     exactly the table lookup for all 16 packed columns at once.
     Binary column pairs pack as lo + 248*hi ({0,1,248,249} all
     bf16-exact; per-batch lo-sums <= 168 < 248 so sums unpack exactly).
  3. ACT copies vals to SBUF bf16; DVE multiplies by an amounts-or-ones
     tile (amounts for the 5 continuous slots, 1.0 for binary slots) so
     one product + one food-axis reduce yields both the nutrition sums
     and the binary count sums; small strided reduces produce per-batch
     / per-day / per-meal sums.
  4. ACT handles tanh/relu/exp/square/abs (penalties, huber, prefs).
  5. All per-batch terms are written into one valcat tile, multiplied by
     a host-built per-(partition, column) mask-weight tile, and
     contracted to a scalar with a single ones-column matmul.
Host work is layout-only: de-interleave ids/amounts, replicate across
partition groups, pack the constant tables, sum 8 per-core partials.
"""

import numpy as np
import ml_dtypes

import concourse.bass as bass
import concourse.tile as tile
from concourse import bacc, mybir

AF = mybir.ActivationFunctionType
OP = mybir.AluOpType
AX = mybir.AxisListType
F32 = mybir.dt.float32
BF16 = mybir.dt.bfloat16
BFNP = ml_dtypes.bfloat16

NCORES = 8
BG = 512            # global batch
BL = BG // NCORES   # 64 batches per core
S = 168             # slots per batch (7 days * 3 meals * 8 foods)
NG = 8              # streams (one per 16-partition group)
NB = BL // NG       # 8 batches per stream
L = NB * S          # 1344 tokens per stream per id-type
L2 = 2 * L          # true tokens ‖ pred tokens
NK = 14             # id planes: 14*16 = 224 >= 223
SIG = 248.0         # binary pair packing scale: lo + 248*hi
MAGIC = 8388608.0   # 2^23 round-half-even trick
ZCONST = 3000.0 * 504.0 / 8.0   # per-core constant part of zeros penalty
CHUNK = 448         # PE moving-operand chunk (1344 = 3*448)

W_HUB = 1.0 / (100.0 * 512.0)
W_PA = 100.0 / 512.0
NV = 92             # valcat columns (91 used + pad to even)


def _build(tc, xp, xt, amtb, am8, wts, iotab, mcat, out):
    import contextlib

    nc = tc.nc

    with contextlib.ExitStack() as ctx:
        sb = ctx.enter_context(tc.tile_pool(name="sb", bufs=1))
        ps = ctx.enter_context(tc.tile_pool(name="ps", bufs=1, space="PSUM"))

        # ---- input DMAs, spread across engine queues by need-time ----
        # sync ring: iota (tiny, unblocks DVE), true ids, raw pred ids
        iotab_s = sb.tile([128, NK], F32, tag="iotab_s")
        nc.sync.dma_start(out=iotab_s[:], in_=iotab)
        xcat = sb.tile([128, L2], BF16, tag="xcat")
        nc.sync.dma_start(out=xcat[:, 0:L], in_=xt)
        xp_s = sb.tile([128, L], F32, tag="xp_s")
        nc.sync.dma_start(out=xp_s[:], in_=xp)
        # scalar ring: lookup weights (PE needs ~first), amounts, tanh input
        wts_s = sb.tile([128, NK * 128 + 1], BF16, tag="wts_s")
        nc.scalar.dma_start(out=wts_s[:], in_=wts)
        amtb_s = sb.tile([128, L2], BF16, tag="amtb_s")
        nc.scalar.dma_start(out=amtb_s[:], in_=amtb)
        am8_s = sb.tile([8, L], F32, tag="am8_s")
        nc.scalar.dma_start(out=am8_s[:], in_=am8)
        # gpsimd ring: mask weights (needed late) + constants
        mcat_s = sb.tile([128, NV], F32, tag="mcat_s")
        nc.gpsimd.dma_start(out=mcat_s[:], in_=mcat)
        valcat = sb.tile([128, NV], F32, tag="valcat")
        nc.gpsimd.memset(valcat[:], 0.0)
        cm222 = sb.tile([128, 1], F32, tag="cm222")
        nc.gpsimd.memset(cm222[:], -222.0)
        cm1680 = sb.tile([128, 1], F32, tag="cm1680")
        nc.gpsimd.memset(cm1680[:], -1680.0)
        ones_t = sb.tile([128, 1], F32, tag="ones_t")
        nc.gpsimd.memset(ones_t[:], 1.0)

        # ---- id planes: oh_k[(g,j), s] = [x == 16k + j]  (bf16 4x) ----
        oh = [
            sb.tile([128, L2], BF16, name=f"oh{k}", tag=f"oh{k}")
            for k in range(NK)
        ]
        for k in range(NK):   # true half first: no preprocessing needed
            nc.vector.tensor_scalar(
                out=oh[k][:, 0:L], in0=xcat[:, 0:L],
                scalar1=iotab_s[:, k:k + 1], scalar2=None, op0=OP.is_equal,
            )
        # pred ids: round-half-even then mask >222.5 -> 0, write bf16
        kp = sb.tile([128, L], F32, tag="kp")
        nc.vector.tensor_scalar(
            out=kp[:], in0=xp_s[:], scalar1=MAGIC, scalar2=MAGIC,
            op0=OP.add, op1=OP.subtract,
        )
        nc.vector.scalar_tensor_tensor(
            out=xcat[:, L:L2], in0=kp[:], scalar=222.5, in1=kp[:],
            op0=OP.is_le, op1=OP.mult,
        )
        for k in range(NK):
            nc.vector.tensor_scalar(
                out=oh[k][:, L:L2], in0=xcat[:, L:L2],
                scalar1=iotab_s[:, k:k + 1], scalar2=None, op0=OP.is_equal,
            )

        # ---- PE: accumulate 14 block-diag lookup matmuls per half ----
        # k outer so one LDWEIGHTS feeds all 3 bank-aligned chunk groups
        vals_t = ps.tile([128, 1536], F32, tag="vals_t")
        vals_p = ps.tile([128, 1536], F32, tag="vals_p")
        for h, v_t in ((0, vals_t), (1, vals_p)):
            for k in range(NK):
                for c0 in (0, 512, 1024):
                    w = min(512, L - c0)
                    nc.tensor.matmul(
                        v_t[:, c0:c0 + w],
                        wts_s[:, 128 * k:128 * (k + 1)],
                        oh[k][:, h * L + c0:h * L + c0 + w],
                        start=(k == 0), stop=(k == NK - 1),
                    )

        # ---- ACT penalties from raw ids/amounts (independent path) ----
        th1 = sb.tile([128, L], F32, tag="th1")
        nc.scalar.activation(
            out=th1[:], in_=xp_s[:], func=AF.Tanh, scale=2.0,
            accum_out=valcat[:, 88:89],
        )
        rl1 = sb.tile([128, L], F32, tag="rl1")
        nc.scalar.activation(
            out=rl1[:], in_=xp_s[:], func=AF.Relu, bias=cm222[:], scale=1.0,
            accum_out=valcat[:, 89:90],
        )
        th2 = sb.tile([8, L], F32, tag="th2")
        nc.scalar.activation(
            out=th2[:], in_=am8_s[:], func=AF.Tanh, scale=2.0,
            accum_out=valcat[0:8, 90:91],
        )

        # ---- per-half: ACT copy PSUM->SBUF bf16, products, food-reduce ----
        vals_sb = sb.tile([128, L2], BF16, tag="vals_sb")
        prdv = sb.tile([128, L2], BF16, tag="prdv")
        prd8 = sb.tile([128, 336], F32, tag="prd8")  # (h, b, d, m)
        for h, v_t in ((0, vals_t), (1, vals_p)):
            cs = slice(h * L, (h + 1) * L)
            nc.scalar.activation(
                out=vals_sb[:, cs], in_=v_t[:, 0:L], func=AF.Copy, scale=1.0,
            )
            nc.vector.tensor_tensor(
                out=prdv[:, cs], in0=vals_sb[:, cs], in1=amtb_s[:, cs],
                op=OP.mult,
            )
            nc.vector.tensor_reduce(
                out=prd8[:, h * 168:(h + 1) * 168],
                in_=prdv[:, cs].rearrange("p (u f) -> p u f", f=8),
                axis=AX.X, op=OP.add,
            )

        # ---- second-stage reduces ----
        psums = sb.tile([128, 16], F32, tag="psums")  # (h, b)
        nc.vector.tensor_reduce(
            out=psums[:], in_=prd8[:].rearrange("p (hb u) -> p hb u", u=21),
            axis=AX.X, op=OP.add,
        )
        meal = sb.tile([128, 48], F32, tag="meal")    # (h, b, m)
        nc.vector.tensor_reduce(
            out=meal[:].rearrange("p (hb m) -> p hb m", m=3),
            in_=prd8[:].rearrange("p (hb d m) -> p hb m d", d=7, m=3),
            axis=AX.X, op=OP.add,
        )
        day = sb.tile([128, 56], F32, tag="day")      # (b, d) pred half
        nc.vector.tensor_reduce(
            out=day[:].rearrange("p (b d) -> p b d", d=7),
            in_=prd8[:, 168:336].rearrange("p (b d m) -> p b d m", d=7, m=3),
            axis=AX.X, op=OP.add,
        )

        # ---- day-level variance: var = s2/7 - (s1/700)^2, cal = day/100 ----
        sq = sb.tile([128, 56], F32, tag="sq")
        nc.scalar.activation(out=sq[:], in_=day[:], func=AF.Square, scale=0.01)
        s2 = sb.tile([128, 8], F32, tag="s2")
        nc.vector.tensor_reduce(
            out=s2[:], in_=sq[:].rearrange("p (b d) -> p b d", d=7),
            axis=AX.X, op=OP.add,
        )
        mu2 = sb.tile([128, 8], F32, tag="mu2")
        nc.vector.scalar_tensor_tensor(
            out=mu2[:], in0=psums[:, 8:16], scalar=1.0 / 490000.0,
            in1=psums[:, 8:16], op0=OP.mult, op1=OP.mult,
        )
        nc.vector.scalar_tensor_tensor(
            out=valcat[:, 80:88], in0=s2[:], scalar=1.0 / 7.0, in1=mu2[:],
            op0=OP.mult, op1=OP.subtract,
        )

        # ---- unpack binary sums: S = lo + 248*hi -> PG[(G lo|hi, P lo|hi)] ----
        # PG cols: 0:8 lot, 8:16 hit, 16:24 lop, 24:32 hip
        pg = sb.tile([128, 32], F32, tag="pg")
        t1 = sb.tile([128, 16], F32, tag="t1")
        nc.vector.tensor_scalar(
            out=t1[:], in0=psums[:], scalar1=1.0 / SIG, scalar2=MAGIC - 0.33871,
            op0=OP.mult, op1=OP.add,
        )
        hi_v = pg[:].rearrange("p (v q b) -> p v q b", v=2, q=2)[:, :, 1:2, :]
        lo_v = pg[:].rearrange("p (v q b) -> p v q b", v=2, q=2)[:, :, 0:1, :]
        nc.vector.tensor_scalar(
            out=hi_v, in0=t1[:], scalar1=MAGIC, scalar2=None, op0=OP.subtract,
        )
        nc.vector.scalar_tensor_tensor(
            out=lo_v, in0=hi_v, scalar=-SIG, in1=psums[:],
            op0=OP.mult, op1=OP.add,
        )
        g2 = pg[:, 0:16]   # gold (true):  lot ‖ hit
        p2 = pg[:, 16:32]  # pred:         lop ‖ hip

        # ---- huber terms -> valcat[0:48] ----
        # d1: nutrition diffs (8) ‖ meal diffs (24), scale 1/700
        d1 = sb.tile([128, 32], F32, tag="d1")
        nc.vector.tensor_tensor(
            out=d1[:, 0:8], in0=psums[:, 8:16], in1=psums[:, 0:8],
            op=OP.subtract,
        )
        nc.vector.tensor_tensor(
            out=d1[:, 8:32], in0=meal[:, 24:48], in1=meal[:, 0:24],
            op=OP.subtract,
        )
        d2 = sb.tile([128, 16], F32, tag="d2")
        nc.vector.tensor_tensor(
            out=d2[:], in0=p2, in1=g2, op=OP.subtract,
        )

        def huber(dst, d_ap, scale, w, tag):
            a_t = sb.tile([128, w], F32, tag=tag + "_a")
            nc.scalar.activation(out=a_t[:], in_=d_ap, func=AF.Abs, scale=scale)
            m_t = sb.tile([128, w], F32, tag=tag + "_m")
            nc.vector.tensor_scalar(
                out=m_t[:], in0=a_t[:], scalar1=1.0, scalar2=None, op0=OP.min
            )
            t_t = sb.tile([128, w], F32, tag=tag + "_t")
            nc.vector.scalar_tensor_tensor(
                out=t_t[:], in0=m_t[:], scalar=-0.5, in1=a_t[:],
                op0=OP.mult, op1=OP.add,
            )
            nc.vector.tensor_tensor(out=dst, in0=m_t[:], in1=t_t[:], op=OP.mult)

        huber(valcat[:, 0:32], d1[:], 1.0 / 700.0, 32, "h1")
        huber(valcat[:, 32:48], d2[:], 1.0, 16, "h2")

        # ---- pref/allergen terms -> valcat[48:80] ----
        gc = sb.tile([128, 16], F32, tag="gc")
        nc.vector.tensor_scalar(
            out=gc[:], in0=g2, scalar1=168.0, scalar2=None, op0=OP.min
        )
        e1 = sb.tile([128, 16], F32, tag="e1")
        nc.scalar.activation(
            out=e1[:], in_=gc[:], func=AF.Exp, scale=10.0, bias=cm1680[:]
        )
        p1 = sb.tile([128, 16], F32, tag="p1")
        nc.vector.tensor_scalar(
            out=p1[:], in0=p2, scalar1=-1.0, scalar2=168.0,
            op0=OP.mult, op1=OP.add,
        )
        q1 = sb.tile([128, 16], F32, tag="q1")
        nc.scalar.activation(out=q1[:], in_=p1[:], func=AF.Square)
        nc.vector.tensor_tensor(
            out=valcat[:, 48:64], in0=e1[:], in1=q1[:], op=OP.mult
        )
        gp = sb.tile([128, 16], F32, tag="gp")
        nc.vector.tensor_scalar(
            out=gp[:], in0=g2, scalar1=0.0, scalar2=None, op0=OP.max
        )
        e2 = sb.tile([128, 16], F32, tag="e2")
        nc.scalar.activation(out=e2[:], in_=gp[:], func=AF.Exp, scale=-10.0)
        q2 = sb.tile([128, 16], F32, tag="q2")
        nc.scalar.activation(out=q2[:], in_=p2, func=AF.Square)
        nc.vector.tensor_tensor(
            out=valcat[:, 64:80], in0=e2[:], in1=q2[:], op=OP.mult
        )

        # ---- weighted contraction: one mult + one ones-column matmul ----
        wv = sb.tile([128, NV], F32, tag="wv")
        nc.vector.tensor_tensor(
            out=wv[:], in0=valcat[:], in1=mcat_s[:], op=OP.mult
        )
        fps = ps.tile([1, NV], F32, tag="fps")
        nc.tensor.matmul(
            fps[:], ones_t[:], wv[:], start=True, stop=True,
        )
        loss_t = sb.tile([1, 1], F32, tag="loss_t")
        nc.vector.tensor_reduce(out=loss_t[:], in_=fps[:], axis=AX.X, op=OP.add)
        lossf = sb.tile([1, 1], F32, tag="lossf")
        nc.vector.tensor_scalar_add(out=lossf[:], in0=loss_t[:], scalar1=ZCONST)
        nc.sync.dma_start(out=out, in_=lossf[:])


def build_program():
    nc = bacc.Bacc("TRN2", target_bir_lowering=False, num_devices=NCORES)
    xp = nc.dram_tensor("xp", [128, L], F32, kind="ExternalInput")
    xt = nc.dram_tensor("xt", [128, L], BF16, kind="ExternalInput")
    amtb = nc.dram_tensor("amtb", [128, L2], BF16, kind="ExternalInput")
    am8 = nc.dram_tensor("am8", [8, L], F32, kind="ExternalInput")
    wts = nc.dram_tensor("wts", [128, NK * 128 + 1], BF16, kind="ExternalInput")
    iotab = nc.dram_tensor("iotab", [128, NK], F32, kind="ExternalInput")
    mcat = nc.dram_tensor("mcat", [128, NV], F32, kind="ExternalInput")
    out = nc.dram_tensor("o", [1, 1], F32, kind="ExternalOutput")
    with tile.TileContext(nc) as tc:
        _build(
            tc, xp.ap(), xt.ap(), amtb.ap(), am8.ap(), wts.ap(),
            iotab.ap(), mcat.ap(), out.ap(),
        )
    nc.compile()
    return nc


def make_const_inputs(data):
    """Host-side constant tables shared by all cores (layout only)."""
    data = np.asarray(data, dtype=np.float32)
    # packed table [224, 16]: 5 cont cols, 7 sigma-packed binary pairs
    pk = np.zeros((224, 16), np.float32)
    pk[:223, 0:5] = data[:, 0:5]
    for u in range(7):
        pk[:223, 5 + u] = data[:, 5 + 2 * u] + SIG * data[:, 6 + 2 * u]
    # 14 block-diagonal lhsT planes + trailing ones column
    wts = np.zeros((128, NK * 128 + 1), BFNP)
    for k in range(NK):
        blk = np.zeros((128, 128), np.float32)
        e_k = pk[16 * k:16 * k + 16, :]
        for g in range(NG):
            blk[16 * g:16 * g + 16, 16 * g:16 * g + 16] = e_k
        wts[:, 128 * k:128 * (k + 1)] = blk.astype(BFNP)
    wts[:, NK * 128] = np.float32(1.0)
    # iota planes: value 16k + (p % 16)
    iotab = (
        16.0 * np.arange(NK)[None, :] + (np.arange(128) % 16)[:, None]
    ).astype(np.float32)
    # mask-weight tile [128, NV]
    c = np.arange(128) % 16
    m = np.zeros((128, NV), np.float32)
    m[:, 0:8] = ((c < 5) * W_HUB)[:, None]            # nutrition huber
    m[:, 8:32] = ((c == 0) * W_HUB)[:, None]          # meal huber
    m[:, 32:40] = (((c == 10) | (c == 11)) * W_HUB)[:, None]      # ingr lo
    m[:, 40:48] = (((c >= 9) & (c <= 11)) * W_HUB)[:, None]       # ingr hi
    m[:, 48:64] = ((c == 5) * W_PA)[:, None]          # prefs lo+hi
    m[:, 64:72] = (((c >= 6) & (c <= 9)) * W_PA)[:, None]         # alrg lo
    m[:, 72:80] = (((c >= 6) & (c <= 8)) * W_PA)[:, None]         # alrg hi
    m[:, 80:88] = ((c == 0) / 512.0)[:, None]         # day variance
    m[:, 88] = -2.0 * 3000.0 / (512.0 * 16.0)         # tanh(2*pid), 16x repl
    m[:, 89] = 1.0 / (512.0 * 16.0)                   # relu(pid-222), 16x
    m[:, 90] = 0.0
    m[0:8, 90] = -3000.0 / 512.0                      # tanh(2*pamt), compact
    return wts, iotab, m


def make_in_maps(y_pred, y, data):
    y_pred = np.asarray(y_pred, dtype=np.float32)
    y = np.asarray(y, dtype=np.float32)
    wts, iotab, mcat = make_const_inputs(data)
    in_maps = []
    for core in range(NCORES):
        sl = slice(core * BL, (core + 1) * BL)

        def streams(arr, comp):
            # [64, 7, 3, 8] -> [8 streams, 1344] (batch-major within stream)
            return np.ascontiguousarray(
                arr[sl, ..., comp], dtype=np.float32).reshape(NG, L)

        pid = streams(y_pred, 0)
        pam = streams(y_pred, 1)
        tid = streams(y, 0)
        tam = streams(y, 1)
        # replicate ids across the 16 partitions of each group
        xp = np.repeat(pid, 16, axis=0)                      # [128, L] f32
        xt = np.repeat(tid.astype(BFNP), 16, axis=0)         # [128, L] bf16
        # amounts-or-ones: slots c<5 amounts, else 1.0 ; true ‖ pred
        c16 = (np.arange(128) % 16)[:, None]
        amtb = np.concatenate(
            [np.repeat(tam, 16, axis=0), np.repeat(pam, 16, axis=0)], axis=1
        )
        amtb = np.where(c16 < 5, amtb, 1.0).astype(BFNP)     # [128, 2L]
        in_maps.append({
            "xp": xp, "xt": xt, "amtb": amtb,
            "am8": pam, "wts": wts, "iotab": iotab, "mcat": mcat,
        })
    return in_maps


_NC_CACHE = None


def _get_nc():
    global _NC_CACHE
    if _NC_CACHE is None:
        _NC_CACHE = build_program()
    return _NC_CACHE


def run_on_hw(y_pred, y, data, **kwargs):
    from concourse.bass_utils import run_bass_kernel_spmd

    nc = _get_nc()
    in_maps = make_in_maps(y_pred, y, data)
    res = run_bass_kernel_spmd(
        nc, in_maps, core_ids=list(range(NCORES)), **kwargs
    )
    parts = [r["o"][0, 0] for r in res.results]
    return np.float32(np.sum(np.asarray(parts, dtype=np.float32))), res


def kernel(y_pred, y, data):
    return run_on_hw(y_pred, y, data)[0]
